# revision 4
# baseline (speedup 1.0000x reference)
"""Trainium2 Bass kernel for nn_DecoderSplatting.

Contract: kernel(**inputs) takes the FULL unsharded inputs
(raw_gaussians [2,4,14,512,512], extrinsics [2,4,3,4], intrinsics [2,3,3])
and returns the full outputs matching reference():
(means, cov, opacity, rgb, big_loss, small_loss).

Sharding: data-parallel over the 8 (b, v) slices, one per NeuronCore.

All ScalarE ops use the single natural_log_exp_and_others table set
(Exp, Ln, Square, Relu) so no ACT table reloads ever happen:
  sigmoid(x) = 1/(1 + e^-x)      (Exp + DVE fast reciprocal)
  softplus(x) = Ln(e^x + 1)      (Exp + Ln-with-bias)
  1/sqrt(g)   = Exp(-0.5 Ln(g))
"""
import numpy as np

import concourse.bacc as bacc
import concourse.mybir as mybir
from concourse.tile import TileContext
from concourse.bass_utils import run_bass_kernel_spmd

F32 = mybir.dt.float32
ALU = mybir.AluOpType
ACTF = mybir.ActivationFunctionType

P = 128          # partitions
FD = 512         # free-dim chunk size
NCHUNK = 4       # 128*2048 pixels per core, 4 chunks of 512
NPIX = P * 2048  # 262144 pixels per core
NCH_IN = 14
NCH_OUT = 13     # 3 means + 6 cov + 1 opacity + 3 rgb
NACC = 48        # 4 chunks x 12 accumulator columns

G_THR = float(-0.1 * np.log(1e-6))

# const column indices
C_R00, C_R01, C_R10, C_R11, C_R20, C_R21 = 0, 1, 2, 3, 4, 5
C_CM0, C_CM1, C_CM2 = 6, 7, 8          # R[i,2]*f
C_T0, C_T1, C_T2 = 9, 10, 11
C_F2 = 12
C_MULT = 13
C_THRB = 14
C_NTHRB = 15                            # -thrb
C_THRS = 16
C_MC = 17                               # 17..32: Mc row-major
C_HY = 33                               # 33..36: hy per chunk
C_NGTHR = 37
NCOL = 38


def _build_program():
    nc = bacc.Bacc("TRN2", target_bir_lowering=False, debug=False, num_devices=8)

    x = nc.dram_tensor("x", [NCH_IN, NPIX], F32, kind="ExternalInput")
    hx_d = nc.dram_tensor("hx", [P, FD], F32, kind="ExternalInput")
    cst_d = nc.dram_tensor("consts", [P, NCOL], F32, kind="ExternalInput")
    out = nc.dram_tensor("out", [NCH_OUT, NPIX], F32, kind="ExternalOutput")
    acc_d = nc.dram_tensor("acc", [P, NACC], F32, kind="ExternalOutput")

    # [ch, pix] viewed as [p, ch, f-total]
    x_v = x[:].rearrange("c (p f) -> p c f", p=P)
    out_v = out[:].rearrange("c (p f) -> p c f", p=P)

    with TileContext(nc) as tc:
        with (
            tc.tile_pool(name="cpool", bufs=1) as cp,
            tc.tile_pool(name="iop", bufs=2) as iop,
            tc.tile_pool(name="oop", bufs=2) as oop,
            tc.tile_pool(name="wp", bufs=1) as wp,
            tc.tile_pool(name="scr", bufs=2) as sc,
        ):
            cst = cp.tile([P, NCOL], F32, tag="cst")
            nc.sync.dma_start(out=cst[:], in_=cst_d[:])
            hx = cp.tile([P, FD], F32, tag="hx")
            nc.sync.dma_start(out=hx[:], in_=hx_d[:])
            acc = cp.tile([P, NACC], F32, tag="acc")

            def col(j):
                return cst[:, j:j + 1]

            act = nc.scalar.activation
            tt = nc.vector.tensor_tensor
            ts = nc.vector.tensor_scalar
            stt = nc.vector.scalar_tensor_tensor
            recip = nc.vector.reciprocal_approx_fast

            for c in range(NCHUNK):
                in_t = iop.tile([P, NCH_IN, FD], F32, tag="in_t")
                nc.sync.dma_start(
                    out=in_t[:], in_=x_v[:, :, c * FD:(c + 1) * FD]
                )
                out_t = oop.tile([P, NCH_OUT, FD], F32, tag="out_t")

                def ch(k):
                    return in_t[:, k, :]

                def och(k):
                    return out_t[:, k, :]

                def sigmoid_into(dst, src_ap):
                    den = sc.tile([P, FD], F32, tag="den")
                    act(den[:], src_ap, ACTF.Exp, scale=-1.0)
                    ts(den[:], den[:], 1.0, None, op0=ALU.add)
                    recip(out=dst, in_=den[:])

                def softplus_into(dst, src_ap):
                    e = sc.tile([P, FD], F32, tag="den")
                    act(e[:], src_ap, ACTF.Exp)
                    act(dst, e[:], ACTF.Ln, bias=1.0)

                # ---------------- means ----------------
                s0 = wp.tile([P, FD], F32, tag="s0")
                sigmoid_into(s0[:], ch(12))
                s1 = wp.tile([P, FD], F32, tag="s1")
                sigmoid_into(s1[:], ch(13))
                tt(out=s0[:], in0=s0[:], in1=hx[:], op=ALU.add)          # w0
                ts(s1[:], s1[:], col(C_HY + c), None, op0=ALU.add)       # w1
                sq0 = wp.tile([P, FD], F32, tag="sq0")
                act(sq0[:], s0[:], ACTF.Square)
                sq1 = wp.tile([P, FD], F32, tag="sq1")
                act(sq1[:], s1[:], ACTF.Square)
                # nnf = (w0^2 + f^2) + w1^2
                stt(sq0[:], sq0[:], col(C_F2), sq1[:], op0=ALU.add, op1=ALU.add)
                tex = wp.tile([P, FD], F32, tag="tex")
                act(tex[:], ch(3), ACTF.Exp, scale=-1.0)                 # e^-disp
                gg = wp.tile([P, FD], F32, tag="gg")
                act(gg[:], tex[:], ACTF.Square, bias=1.0, scale=0.01)    # ((t+100)/100)^2
                tt(out=gg[:], in0=gg[:], in1=sq0[:], op=ALU.mult)        # g
                act(sq1[:], gg[:], ACTF.Ln)                              # ln g
                act(gg[:], sq1[:], ACTF.Exp, scale=-0.5)                 # g^-1/2
                # sfac = (t+1) * g^-1/2
                stt(sq1[:], tex[:], 1.0, gg[:], op0=ALU.add, op1=ALU.mult)
                tt(out=s0[:], in0=s0[:], in1=sq1[:], op=ALU.mult)        # W0
                tt(out=s1[:], in0=s1[:], in1=sq1[:], op=ALU.mult)        # W1
                for i in range(3):
                    xi = sc.tile([P, FD], F32, tag="xi")
                    ts(xi[:], sq1[:], col(C_CM0 + i), col(C_T0 + i),
                       op0=ALU.mult, op1=ALU.add)
                    stt(xi[:], s0[:], col(C_R00 + 2 * i), xi[:],
                        op0=ALU.mult, op1=ALU.add)
                    stt(och(i), s1[:], col(C_R01 + 2 * i), xi[:],
                        op0=ALU.mult, op1=ALU.add)

                # ---------------- opacity / rgb ----------------
                sigmoid_into(och(9), ch(4))
                for i in range(3):
                    softplus_into(och(10 + i), ch(i))

                # ---------------- scales + losses ----------------
                sp = []
                for k in range(3):
                    spk = wp.tile([P, FD], F32, tag=f"sp{k}")
                    softplus_into(spk[:], ch(5 + k))
                    sp.append(spk)
                a0 = c * 12
                for k in range(3):
                    junk = sc.tile([P, FD], F32, tag="junk")
                    ts(junk[:], sp[k][:], col(C_THRB), 0.0, op0=ALU.is_gt,
                       op1=ALU.add, accum_out=acc[:, a0 + k:a0 + k + 1])
                    junk = sc.tile([P, FD], F32, tag="junk")
                    act(junk[:], sp[k][:], ACTF.Relu, bias=col(C_NTHRB),
                        accum_out=acc[:, a0 + 3 + k:a0 + 4 + k])
                    junk = sc.tile([P, FD], F32, tag="junk")
                    ts(junk[:], sp[k][:], col(C_THRS), 0.0, op0=ALU.is_lt,
                       op1=ALU.add, accum_out=acc[:, a0 + 6 + k:a0 + 7 + k])
                    lnt = sc.tile([P, FD], F32, tag="lnt")
                    act(lnt[:], sp[k][:], ACTF.Ln, scale=col(C_MULT))
                    junk = sc.tile([P, FD], F32, tag="junk")
                    act(junk[:], lnt[:], ACTF.Relu, bias=col(C_NGTHR), scale=-0.1,
                        accum_out=acc[:, a0 + 9 + k:a0 + 10 + k])

                # ---------------- quaternion -> q' = Mc q ----------------
                qp = []
                for a in range(4):
                    qpa = wp.tile([P, FD], F32, tag=f"qp{a}")
                    ts(qpa[:], ch(8), col(C_MC + 4 * a), None, op0=ALU.mult)
                    for bq in range(1, 4):
                        stt(qpa[:], ch(8 + bq), col(C_MC + 4 * a + bq), qpa[:],
                            op0=ALU.mult, op1=ALU.add)
                    qp.append(qpa)

                # squares
                sqq = []
                for a in range(4):
                    t = wp.tile([P, FD], F32, tag=f"qq{a}")
                    act(t[:], qp[a][:], ACTF.Square)
                    sqq.append(t)
                A, B, C_, D = sqq

                # doubled cross products
                d2 = sc.tile([P, FD], F32, tag="d2")
                ts(d2[:], qp[0][:], 2.0, None, op0=ALU.mult)
                ab = wp.tile([P, FD], F32, tag="ab")
                tt(out=ab[:], in0=d2[:], in1=qp[1][:], op=ALU.mult)
                acx = wp.tile([P, FD], F32, tag="acx")
                tt(out=acx[:], in0=d2[:], in1=qp[2][:], op=ALU.mult)
                ad = wp.tile([P, FD], F32, tag="ad")
                tt(out=ad[:], in0=d2[:], in1=qp[3][:], op=ALU.mult)
                d2 = sc.tile([P, FD], F32, tag="d2")
                ts(d2[:], qp[1][:], 2.0, None, op0=ALU.mult)
                bc = wp.tile([P, FD], F32, tag="bc")
                tt(out=bc[:], in0=d2[:], in1=qp[2][:], op=ALU.mult)
                bd = wp.tile([P, FD], F32, tag="bd")
                tt(out=bd[:], in0=d2[:], in1=qp[3][:], op=ALU.mult)
                d2 = sc.tile([P, FD], F32, tag="d2")
                ts(d2[:], qp[2][:], 2.0, None, op0=ALU.mult)
                cd = wp.tile([P, FD], F32, tag="cd")
                tt(out=cd[:], in0=d2[:], in1=qp[3][:], op=ALU.mult)

                # off-diagonal P entries.  P10 overwrites ab, P20 overwrites
                # acx, P21 overwrites bc (their last reads happen first).
                P01 = wp.tile([P, FD], F32, tag="P01")
                tt(out=P01[:], in0=ab[:], in1=cd[:], op=ALU.subtract)
                tt(out=ab[:], in0=ab[:], in1=cd[:], op=ALU.add)            # P10
                P02 = wp.tile([P, FD], F32, tag="P02")
                tt(out=P02[:], in0=acx[:], in1=bd[:], op=ALU.add)
                tt(out=acx[:], in0=acx[:], in1=bd[:], op=ALU.subtract)     # P20
                P12 = wp.tile([P, FD], F32, tag="P12")
                tt(out=P12[:], in0=bc[:], in1=ad[:], op=ALU.subtract)
                tt(out=bc[:], in0=bc[:], in1=ad[:], op=ALU.add)            # P21
                P10, P20, P21 = ab, acx, bc

                # diagonal: u=A+D (into D), vv=B+C (into cd), n2e (into ad),
                # P00 (into bd), P11 (into C_), P22 (into B)
                tt(out=D[:], in0=A[:], in1=D[:], op=ALU.add)               # u
                tt(out=cd[:], in0=B[:], in1=C_[:], op=ALU.add)             # vv
                stt(ad[:], D[:], 1e-8, cd[:], op0=ALU.add, op1=ALU.add)    # n2e
                stt(bd[:], cd[:], -2.0, ad[:], op0=ALU.mult, op1=ALU.add)  # P00
                tt(out=D[:], in0=A[:], in1=C_[:], op=ALU.add)              # v2
                stt(C_[:], D[:], -2.0, ad[:], op0=ALU.mult, op1=ALU.add)   # P11->C_
                tt(out=D[:], in0=A[:], in1=B[:], op=ALU.add)               # v3
                stt(B[:], D[:], -2.0, ad[:], op0=ALU.mult, op1=ALU.add)    # P22->B
                P00, P11, P22 = bd, C_, B

                # h = 1/n2e (into A), mh = h*mult (in place)
                recip(out=A[:], in_=ad[:])
                ts(A[:], A[:], col(C_MULT), None, op0=ALU.mult)            # mh

                # sz_k = sp_k * mh  (in place into sp_k)
                for k in range(3):
                    tt(out=sp[k][:], in0=sp[k][:], in1=A[:], op=ALU.mult)

                # Y[i][k] = P[i][k] * sz_k  (in place into P tiles)
                Pm = [[P00, P01, P02], [P10, P11, P12], [P20, P21, P22]]
                for i in range(3):
                    for k in range(3):
                        tt(out=Pm[i][k][:], in0=Pm[i][k][:], in1=sp[k][:],
                           op=ALU.mult)
                Y = Pm

                # cov output channel map: 3:c00 4:c01 5:c02 6:c11 7:c12 8:c22
                # diagonals via ACT squares
                for i, oc in ((0, 3), (1, 6), (2, 8)):
                    ya = sc.tile([P, FD], F32, tag="ya")
                    act(ya[:], Y[i][0][:], ACTF.Square)
                    yb = sc.tile([P, FD], F32, tag="yb")
                    act(yb[:], Y[i][1][:], ACTF.Square)
                    yc = sc.tile([P, FD], F32, tag="yc")
                    act(yc[:], Y[i][2][:], ACTF.Square)
                    tt(out=ya[:], in0=ya[:], in1=yb[:], op=ALU.add)
                    tt(out=och(oc), in0=ya[:], in1=yc[:], op=ALU.add)
                # off-diagonals
                for i, l, oc in ((0, 1, 4), (0, 2, 5), (1, 2, 7)):
                    e1 = sc.tile([P, FD], F32, tag="e1")
                    tt(out=e1[:], in0=Y[i][0][:], in1=Y[l][0][:], op=ALU.mult)
                    e2 = sc.tile([P, FD], F32, tag="e2")
                    tt(out=e2[:], in0=Y[i][1][:], in1=Y[l][1][:], op=ALU.mult)
                    tt(out=e1[:], in0=e1[:], in1=e2[:], op=ALU.add)
                    tt(out=e2[:], in0=Y[i][2][:], in1=Y[l][2][:], op=ALU.mult)
                    tt(out=och(oc), in0=e1[:], in1=e2[:], op=ALU.add)

                nc.sync.dma_start(
                    out=out_v[:, :, c * FD:(c + 1) * FD], in_=out_t[:]
                )

            nc.sync.dma_start(out=acc_d[:], in_=acc[:])

    nc.compile()
    return nc


_CACHED_NC = None


def get_program():
    global _CACHED_NC
    if _CACHED_NC is None:
        _CACHED_NC = _build_program()
    return _CACHED_NC


def _rot_to_quat(R):
    R = R.astype(np.float64)
    t = np.trace(R)
    if t > 0:
        s = np.sqrt(t + 1.0) * 2
        w = 0.25 * s
        xq = (R[2, 1] - R[1, 2]) / s
        y = (R[0, 2] - R[2, 0]) / s
        z = (R[1, 0] - R[0, 1]) / s
    else:
        i = int(np.argmax(np.diag(R)))
        if i == 0:
            s = np.sqrt(1.0 + R[0, 0] - R[1, 1] - R[2, 2]) * 2
            xq = 0.25 * s
            w = (R[2, 1] - R[1, 2]) / s
            y = (R[0, 1] + R[1, 0]) / s
            z = (R[0, 2] + R[2, 0]) / s
        elif i == 1:
            s = np.sqrt(1.0 - R[0, 0] + R[1, 1] - R[2, 2]) * 2
            y = 0.25 * s
            w = (R[0, 2] - R[2, 0]) / s
            xq = (R[0, 1] + R[1, 0]) / s
            z = (R[1, 2] + R[2, 1]) / s
        else:
            s = np.sqrt(1.0 - R[0, 0] - R[1, 1] + R[2, 2]) * 2
            z = 0.25 * s
            w = (R[1, 0] - R[0, 1]) / s
            xq = (R[0, 2] + R[2, 0]) / s
            y = (R[1, 2] + R[2, 1]) / s
    q = np.array([xq, y, z, w])
    return q / np.linalg.norm(q)


def _quat_lmul(qc):
    xq, y, z, w = qc
    return np.array([
        [w, -z, y, xq],
        [z, w, -xq, y],
        [-y, xq, w, z],
        [-xq, -y, -z, w],
    ])


def _make_consts(K, R, t):
    f = np.float64(K[0, 0])
    mult = np.float64(np.linalg.inv(K[:2, :2].astype(np.float64)).sum())
    qc = _rot_to_quat(R)
    Mc = _quat_lmul(qc)
    cols = np.zeros(NCOL, np.float64)
    cols[C_R00], cols[C_R01] = R[0, 0], R[0, 1]
    cols[C_R10], cols[C_R11] = R[1, 0], R[1, 1]
    cols[C_R20], cols[C_R21] = R[2, 0], R[2, 1]
    for i in range(3):
        cols[C_CM0 + i] = np.float64(R[i, 2]) * f
        cols[C_T0 + i] = t[i]
    cols[C_F2] = f * f
    cols[C_MULT] = mult
    thrb = np.float32(0.05 / np.float32(mult))
    thrs = np.float32(1e-6 / np.float32(mult))
    cols[C_THRB] = thrb
    cols[C_NTHRB] = -thrb
    cols[C_THRS] = thrs
    cols[C_NGTHR] = -G_THR
    for a in range(4):
        for bq in range(4):
            cols[C_MC + 4 * a + bq] = Mc[a, bq]
    cst = np.tile(cols.astype(np.float32)[None, :], (P, 1))
    # hy columns vary per partition: hy[p, c] = 4p + c - 256.5
    pp = np.arange(P, dtype=np.float32)
    for c in range(NCHUNK):
        cst[:, C_HY + c] = 4.0 * pp + c - 256.5
    return cst, np.float32(mult)


def kernel(raw_gaussians, extrinsics, intrinsics):
    b_, v_, c_, h_, w_ = raw_gaussians.shape
    assert (b_, v_, c_, h_, w_) == (2, 4, 14, 512, 512)
    nc = get_program()

    hx = np.tile(
        (np.arange(FD, dtype=np.float32) - 256.5)[None, :], (P, 1)
    ).astype(np.float32)

    in_maps = []
    mults = []
    for m in range(8):
        b, v = m // 4, m % 4
        cst, mult = _make_consts(
            np.asarray(intrinsics[b], np.float32),
            np.asarray(extrinsics[b, v, :3, :3], np.float32),
            np.asarray(extrinsics[b, v, :3, 3], np.float32),
        )
        mults.append(mult)
        xm = np.ascontiguousarray(
            np.asarray(raw_gaussians[b, v], np.float32).reshape(NCH_IN, NPIX)
        )
        in_maps.append({"x": xm, "hx": hx, "consts": cst})

    res = run_bass_kernel_spmd(nc, in_maps, core_ids=list(range(8)))

    means = np.empty((2, 4, 512, 512, 3), np.float32)
    cov = np.empty((2, 4, 512, 512, 3, 3), np.float32)
    opacity = np.empty((2, 4, 512, 512, 1), np.float32)
    rgb = np.empty((2, 4, 512, 512, 3), np.float32)
    big_num = 0.0
    big_cnt = 0.0
    small_num = 0.0
    small_cnt = 0.0
    covmap = {(0, 0): 3, (0, 1): 4, (0, 2): 5, (1, 1): 6, (1, 2): 7, (2, 2): 8}
    for m in range(8):
        b, v = m // 4, m % 4
        o = res.results[m]["out"].reshape(NCH_OUT, 512, 512)
        means[b, v] = np.moveaxis(o[0:3], 0, -1)
        for (i, l), ocn in covmap.items():
            cov[b, v, :, :, i, l] = o[ocn]
            if i != l:
                cov[b, v, :, :, l, i] = o[ocn]
        opacity[b, v] = o[9][..., None]
        rgb[b, v] = np.moveaxis(o[10:13], 0, -1)

        a = res.results[m]["acc"].reshape(P, NCHUNK, 12).sum(axis=(0, 1))
        mult = np.float64(mults[m])
        big_cnt += a[0] + a[1] + a[2]
        big_num += mult * (a[3] + a[4] + a[5]) + 0.05 * (a[0] + a[1] + a[2])
        small_cnt += a[6] + a[7] + a[8]
        small_num += (a[9] + a[10] + a[11]) + G_THR * (a[6] + a[7] + a[8])

    big_loss = np.float32(big_num / big_cnt) if big_cnt > 0 else np.float32(0.0)
    small_loss = (
        np.float32(small_num / small_cnt) if small_cnt > 0 else np.float32(0.0)
    )
    return means, cov, opacity, rgb, big_loss, small_loss


# revision 7
# speedup vs baseline: 1.1845x; 1.1845x over previous
"""Trainium2 Bass kernel for nn_DecoderSplatting.

Contract: kernel(**inputs) takes the FULL unsharded inputs
(raw_gaussians [2,4,14,512,512], extrinsics [2,4,3,4], intrinsics [2,3,3])
and returns the full outputs matching reference():
(means, cov, opacity, rgb, big_loss, small_loss).

Sharding: data-parallel over the 8 (b, v) slices, one per NeuronCore.

All ScalarE ops use the single natural_log_exp_and_others table set
(Exp, Ln, Square, Relu) so no ACT table reloads ever happen:
  sigmoid(x) = 1/(1 + e^-x)      (Exp + DVE fast reciprocal)
  softplus(x) = Ln(e^x + 1)      (Exp + Ln-with-bias)
  1/sqrt(g)   = Exp(-0.5 Ln(g))
"""
import numpy as np

import concourse.bacc as bacc
import concourse.mybir as mybir
from concourse.tile import TileContext
from concourse.bass_utils import run_bass_kernel_spmd

F32 = mybir.dt.float32
ALU = mybir.AluOpType
ACTF = mybir.ActivationFunctionType

P = 128          # partitions
FD = 512         # free-dim chunk size
NCHUNK = 4       # 128*2048 pixels per core, 4 chunks of 512
NPIX = P * 2048  # 262144 pixels per core
NCH_IN = 14
NCH_OUT = 13     # 3 means + 6 cov + 1 opacity + 3 rgb
NACC = 48        # 4 chunks x 12 accumulator columns

G_THR = float(-0.1 * np.log(1e-6))

# const column indices
C_R00, C_R01, C_R10, C_R11, C_R20, C_R21 = 0, 1, 2, 3, 4, 5
C_CM0, C_CM1, C_CM2 = 6, 7, 8          # R[i,2]*f
C_T0, C_T1, C_T2 = 9, 10, 11
C_F2 = 12
C_MULT = 13
C_THRB = 14
C_NTHRB = 15                            # -thrb
C_THRS = 16
C_MC = 17                               # 17..32: Mc row-major
C_HY = 33                               # 33..36: hy per chunk
C_NGTHR = 37
NCOL = 38


class _BaccOneActTable(bacc.Bacc):
    """Restrict ACT table selection to the single set containing every
    activation function this kernel uses (Exp, Ln, Square, Relu), so the
    table-load pass emits exactly one load instead of thrashing."""

    def insert_act_table_loads(self):
        import bass_rust as _br
        from concourse.hw_specs import get_activation_tables
        tables = [
            (n, (f if n == "natural_log_exp_and_others" else set()))
            for n, f in get_activation_tables(self.m.arch).items()
        ]
        _br.insert_act_table_loads(self, tables)


def _build_program():
    nc = _BaccOneActTable("TRN2", target_bir_lowering=False, debug=False, num_devices=8)

    x = nc.dram_tensor("x", [NCH_IN, NPIX], F32, kind="ExternalInput")
    hx_d = nc.dram_tensor("hx", [P, FD], F32, kind="ExternalInput")
    cst_d = nc.dram_tensor("consts", [P, NCOL], F32, kind="ExternalInput")
    out = nc.dram_tensor("out", [NCH_OUT, NPIX], F32, kind="ExternalOutput")
    acc_d = nc.dram_tensor("acc", [P, NACC], F32, kind="ExternalOutput")

    # [ch, pix] viewed as [p, ch, f-total]
    x_v = x[:].rearrange("c (p f) -> p c f", p=P)
    out_v = out[:].rearrange("c (p f) -> p c f", p=P)

    with TileContext(nc) as tc:
        with (
            tc.tile_pool(name="cpool", bufs=1) as cp,
            tc.tile_pool(name="iop", bufs=2) as iop,
            tc.tile_pool(name="oop", bufs=2) as oop,
            tc.tile_pool(name="wp", bufs=1) as wp,
            tc.tile_pool(name="scr", bufs=2) as sc,
        ):
            cst = cp.tile([P, NCOL], F32, tag="cst")
            nc.sync.dma_start(out=cst[:], in_=cst_d[:])
            hx = cp.tile([P, FD], F32, tag="hx")
            nc.sync.dma_start(out=hx[:], in_=hx_d[:])
            acc = cp.tile([P, NACC], F32, tag="acc")

            def col(j):
                return cst[:, j:j + 1]

            act = nc.scalar.activation
            tt = nc.vector.tensor_tensor
            ts = nc.vector.tensor_scalar
            stt = nc.vector.scalar_tensor_tensor
            recip = nc.vector.reciprocal_approx_fast

            for c in range(NCHUNK):
                in_t = iop.tile([P, NCH_IN, FD], F32, tag="in_t")
                nc.sync.dma_start(
                    out=in_t[:], in_=x_v[:, :, c * FD:(c + 1) * FD]
                )
                out_t = oop.tile([P, NCH_OUT, FD], F32, tag="out_t")

                def ch(k):
                    return in_t[:, k, :]

                def och(k):
                    return out_t[:, k, :]

                def sigmoid_into(dst, src_ap):
                    den = sc.tile([P, FD], F32, tag="den")
                    act(den[:], src_ap, ACTF.Exp, scale=-1.0)
                    ts(den[:], den[:], 1.0, None, op0=ALU.add)
                    recip(out=dst, in_=den[:])

                def softplus_into(dst, src_ap):
                    e = sc.tile([P, FD], F32, tag="den")
                    act(e[:], src_ap, ACTF.Exp)
                    act(dst, e[:], ACTF.Ln, bias=1.0)

                # ---------------- means ----------------
                s0 = wp.tile([P, FD], F32, tag="s0")
                sigmoid_into(s0[:], ch(12))
                s1 = wp.tile([P, FD], F32, tag="s1")
                sigmoid_into(s1[:], ch(13))
                tt(out=s0[:], in0=s0[:], in1=hx[:], op=ALU.add)          # w0
                ts(s1[:], s1[:], col(C_HY + c), None, op0=ALU.add)       # w1
                sq0 = wp.tile([P, FD], F32, tag="sq0")
                act(sq0[:], s0[:], ACTF.Square)
                sq1 = wp.tile([P, FD], F32, tag="sq1")
                act(sq1[:], s1[:], ACTF.Square)
                # nnf = (w0^2 + f^2) + w1^2
                stt(sq0[:], sq0[:], col(C_F2), sq1[:], op0=ALU.add, op1=ALU.add)
                tex = wp.tile([P, FD], F32, tag="tex")
                act(tex[:], ch(3), ACTF.Exp, scale=-1.0)                 # e^-disp
                gg = wp.tile([P, FD], F32, tag="gg")
                act(gg[:], tex[:], ACTF.Square, bias=1.0, scale=0.01)    # ((t+100)/100)^2
                tt(out=gg[:], in0=gg[:], in1=sq0[:], op=ALU.mult)        # g
                act(sq1[:], gg[:], ACTF.Ln)                              # ln g
                act(gg[:], sq1[:], ACTF.Exp, scale=-0.5)                 # g^-1/2
                # sfac = (t+1) * g^-1/2
                stt(sq1[:], tex[:], 1.0, gg[:], op0=ALU.add, op1=ALU.mult)
                tt(out=s0[:], in0=s0[:], in1=sq1[:], op=ALU.mult)        # W0
                tt(out=s1[:], in0=s1[:], in1=sq1[:], op=ALU.mult)        # W1
                for i in range(3):
                    xi = sc.tile([P, FD], F32, tag="xi")
                    ts(xi[:], sq1[:], col(C_CM0 + i), col(C_T0 + i),
                       op0=ALU.mult, op1=ALU.add)
                    stt(xi[:], s0[:], col(C_R00 + 2 * i), xi[:],
                        op0=ALU.mult, op1=ALU.add)
                    stt(och(i), s1[:], col(C_R01 + 2 * i), xi[:],
                        op0=ALU.mult, op1=ALU.add)

                # ---------------- opacity / rgb ----------------
                sigmoid_into(och(9), ch(4))
                for i in range(3):
                    softplus_into(och(10 + i), ch(i))

                # ---------------- scales + losses ----------------
                sp = []
                for k in range(3):
                    spk = wp.tile([P, FD], F32, tag=f"sp{k}")
                    softplus_into(spk[:], ch(5 + k))
                    sp.append(spk)
                a0 = c * 12
                for k in range(3):
                    junk = sc.tile([P, FD], F32, tag="junk")
                    ts(junk[:], sp[k][:], col(C_THRB), 0.0, op0=ALU.is_gt,
                       op1=ALU.add, accum_out=acc[:, a0 + k:a0 + k + 1])
                    junk = sc.tile([P, FD], F32, tag="junk")
                    act(junk[:], sp[k][:], ACTF.Relu, bias=col(C_NTHRB),
                        accum_out=acc[:, a0 + 3 + k:a0 + 4 + k])
                    junk = sc.tile([P, FD], F32, tag="junk")
                    ts(junk[:], sp[k][:], col(C_THRS), 0.0, op0=ALU.is_lt,
                       op1=ALU.add, accum_out=acc[:, a0 + 6 + k:a0 + 7 + k])
                    lnt = sc.tile([P, FD], F32, tag="lnt")
                    act(lnt[:], sp[k][:], ACTF.Ln, scale=col(C_MULT))
                    junk = sc.tile([P, FD], F32, tag="junk")
                    act(junk[:], lnt[:], ACTF.Relu, bias=col(C_NGTHR), scale=-0.1,
                        accum_out=acc[:, a0 + 9 + k:a0 + 10 + k])

                # ---------------- quaternion -> q' = Mc q ----------------
                qp = []
                for a in range(4):
                    qpa = wp.tile([P, FD], F32, tag=f"qp{a}")
                    ts(qpa[:], ch(8), col(C_MC + 4 * a), None, op0=ALU.mult)
                    for bq in range(1, 4):
                        stt(qpa[:], ch(8 + bq), col(C_MC + 4 * a + bq), qpa[:],
                            op0=ALU.mult, op1=ALU.add)
                    qp.append(qpa)

                # squares
                sqq = []
                for a in range(4):
                    t = wp.tile([P, FD], F32, tag=f"qq{a}")
                    act(t[:], qp[a][:], ACTF.Square)
                    sqq.append(t)
                A, B, C_, D = sqq

                # doubled cross products
                d2 = sc.tile([P, FD], F32, tag="d2")
                ts(d2[:], qp[0][:], 2.0, None, op0=ALU.mult)
                ab = wp.tile([P, FD], F32, tag="ab")
                tt(out=ab[:], in0=d2[:], in1=qp[1][:], op=ALU.mult)
                acx = wp.tile([P, FD], F32, tag="acx")
                tt(out=acx[:], in0=d2[:], in1=qp[2][:], op=ALU.mult)
                ad = wp.tile([P, FD], F32, tag="ad")
                tt(out=ad[:], in0=d2[:], in1=qp[3][:], op=ALU.mult)
                d2 = sc.tile([P, FD], F32, tag="d2")
                ts(d2[:], qp[1][:], 2.0, None, op0=ALU.mult)
                bc = wp.tile([P, FD], F32, tag="bc")
                tt(out=bc[:], in0=d2[:], in1=qp[2][:], op=ALU.mult)
                bd = wp.tile([P, FD], F32, tag="bd")
                tt(out=bd[:], in0=d2[:], in1=qp[3][:], op=ALU.mult)
                d2 = sc.tile([P, FD], F32, tag="d2")
                ts(d2[:], qp[2][:], 2.0, None, op0=ALU.mult)
                cd = wp.tile([P, FD], F32, tag="cd")
                tt(out=cd[:], in0=d2[:], in1=qp[3][:], op=ALU.mult)

                # off-diagonal P entries.  P10 overwrites ab, P20 overwrites
                # acx, P21 overwrites bc (their last reads happen first).
                P01 = wp.tile([P, FD], F32, tag="P01")
                tt(out=P01[:], in0=ab[:], in1=cd[:], op=ALU.subtract)
                tt(out=ab[:], in0=ab[:], in1=cd[:], op=ALU.add)            # P10
                P02 = wp.tile([P, FD], F32, tag="P02")
                tt(out=P02[:], in0=acx[:], in1=bd[:], op=ALU.add)
                tt(out=acx[:], in0=acx[:], in1=bd[:], op=ALU.subtract)     # P20
                P12 = wp.tile([P, FD], F32, tag="P12")
                tt(out=P12[:], in0=bc[:], in1=ad[:], op=ALU.subtract)
                tt(out=bc[:], in0=bc[:], in1=ad[:], op=ALU.add)            # P21
                P10, P20, P21 = ab, acx, bc

                # diagonal: u=A+D (into D), vv=B+C (into cd), n2e (into ad),
                # P00 (into bd), P11 (into C_), P22 (into B)
                tt(out=D[:], in0=A[:], in1=D[:], op=ALU.add)               # u
                tt(out=cd[:], in0=B[:], in1=C_[:], op=ALU.add)             # vv
                stt(ad[:], D[:], 1e-8, cd[:], op0=ALU.add, op1=ALU.add)    # n2e
                stt(bd[:], cd[:], -2.0, ad[:], op0=ALU.mult, op1=ALU.add)  # P00
                tt(out=D[:], in0=A[:], in1=C_[:], op=ALU.add)              # v2
                stt(C_[:], D[:], -2.0, ad[:], op0=ALU.mult, op1=ALU.add)   # P11->C_
                tt(out=D[:], in0=A[:], in1=B[:], op=ALU.add)               # v3
                stt(B[:], D[:], -2.0, ad[:], op0=ALU.mult, op1=ALU.add)    # P22->B
                P00, P11, P22 = bd, C_, B

                # h = 1/n2e (into A), mh = h*mult (in place)
                recip(out=A[:], in_=ad[:])
                ts(A[:], A[:], col(C_MULT), None, op0=ALU.mult)            # mh

                # sz_k = sp_k * mh  (in place into sp_k)
                for k in range(3):
                    tt(out=sp[k][:], in0=sp[k][:], in1=A[:], op=ALU.mult)

                # Y[i][k] = P[i][k] * sz_k  (in place into P tiles)
                Pm = [[P00, P01, P02], [P10, P11, P12], [P20, P21, P22]]
                for i in range(3):
                    for k in range(3):
                        tt(out=Pm[i][k][:], in0=Pm[i][k][:], in1=sp[k][:],
                           op=ALU.mult)
                Y = Pm

                # cov output channel map: 3:c00 4:c01 5:c02 6:c11 7:c12 8:c22
                # diagonals via ACT squares
                for i, oc in ((0, 3), (1, 6), (2, 8)):
                    ya = sc.tile([P, FD], F32, tag="ya")
                    act(ya[:], Y[i][0][:], ACTF.Square)
                    yb = sc.tile([P, FD], F32, tag="yb")
                    act(yb[:], Y[i][1][:], ACTF.Square)
                    yc = sc.tile([P, FD], F32, tag="yc")
                    act(yc[:], Y[i][2][:], ACTF.Square)
                    tt(out=ya[:], in0=ya[:], in1=yb[:], op=ALU.add)
                    tt(out=och(oc), in0=ya[:], in1=yc[:], op=ALU.add)
                # off-diagonals
                for i, l, oc in ((0, 1, 4), (0, 2, 5), (1, 2, 7)):
                    e1 = sc.tile([P, FD], F32, tag="e1")
                    tt(out=e1[:], in0=Y[i][0][:], in1=Y[l][0][:], op=ALU.mult)
                    e2 = sc.tile([P, FD], F32, tag="e2")
                    tt(out=e2[:], in0=Y[i][1][:], in1=Y[l][1][:], op=ALU.mult)
                    tt(out=e1[:], in0=e1[:], in1=e2[:], op=ALU.add)
                    tt(out=e2[:], in0=Y[i][2][:], in1=Y[l][2][:], op=ALU.mult)
                    tt(out=och(oc), in0=e1[:], in1=e2[:], op=ALU.add)

                nc.sync.dma_start(
                    out=out_v[:, :, c * FD:(c + 1) * FD], in_=out_t[:]
                )

            nc.sync.dma_start(out=acc_d[:], in_=acc[:])

    nc.compile()
    return nc


_CACHED_NC = None
LAST_IN_MAPS = None


def get_program():
    global _CACHED_NC
    if _CACHED_NC is None:
        _CACHED_NC = _build_program()
    return _CACHED_NC


def _rot_to_quat(R):
    R = R.astype(np.float64)
    t = np.trace(R)
    if t > 0:
        s = np.sqrt(t + 1.0) * 2
        w = 0.25 * s
        xq = (R[2, 1] - R[1, 2]) / s
        y = (R[0, 2] - R[2, 0]) / s
        z = (R[1, 0] - R[0, 1]) / s
    else:
        i = int(np.argmax(np.diag(R)))
        if i == 0:
            s = np.sqrt(1.0 + R[0, 0] - R[1, 1] - R[2, 2]) * 2
            xq = 0.25 * s
            w = (R[2, 1] - R[1, 2]) / s
            y = (R[0, 1] + R[1, 0]) / s
            z = (R[0, 2] + R[2, 0]) / s
        elif i == 1:
            s = np.sqrt(1.0 - R[0, 0] + R[1, 1] - R[2, 2]) * 2
            y = 0.25 * s
            w = (R[0, 2] - R[2, 0]) / s
            xq = (R[0, 1] + R[1, 0]) / s
            z = (R[1, 2] + R[2, 1]) / s
        else:
            s = np.sqrt(1.0 - R[0, 0] - R[1, 1] + R[2, 2]) * 2
            z = 0.25 * s
            w = (R[1, 0] - R[0, 1]) / s
            xq = (R[0, 2] + R[2, 0]) / s
            y = (R[1, 2] + R[2, 1]) / s
    q = np.array([xq, y, z, w])
    return q / np.linalg.norm(q)


def _quat_lmul(qc):
    xq, y, z, w = qc
    return np.array([
        [w, -z, y, xq],
        [z, w, -xq, y],
        [-y, xq, w, z],
        [-xq, -y, -z, w],
    ])


def _make_consts(K, R, t):
    f = np.float64(K[0, 0])
    mult = np.float64(np.linalg.inv(K[:2, :2].astype(np.float64)).sum())
    qc = _rot_to_quat(R)
    Mc = _quat_lmul(qc)
    cols = np.zeros(NCOL, np.float64)
    cols[C_R00], cols[C_R01] = R[0, 0], R[0, 1]
    cols[C_R10], cols[C_R11] = R[1, 0], R[1, 1]
    cols[C_R20], cols[C_R21] = R[2, 0], R[2, 1]
    for i in range(3):
        cols[C_CM0 + i] = np.float64(R[i, 2]) * f
        cols[C_T0 + i] = t[i]
    cols[C_F2] = f * f
    cols[C_MULT] = mult
    thrb = np.float32(0.05 / np.float32(mult))
    thrs = np.float32(1e-6 / np.float32(mult))
    cols[C_THRB] = thrb
    cols[C_NTHRB] = -thrb
    cols[C_THRS] = thrs
    cols[C_NGTHR] = -G_THR
    for a in range(4):
        for bq in range(4):
            cols[C_MC + 4 * a + bq] = Mc[a, bq]
    cst = np.tile(cols.astype(np.float32)[None, :], (P, 1))
    # hy columns vary per partition: hy[p, c] = 4p + c - 256.5
    pp = np.arange(P, dtype=np.float32)
    for c in range(NCHUNK):
        cst[:, C_HY + c] = 4.0 * pp + c - 256.5
    return cst, np.float32(mult)


def kernel(raw_gaussians, extrinsics, intrinsics):
    b_, v_, c_, h_, w_ = raw_gaussians.shape
    assert (b_, v_, c_, h_, w_) == (2, 4, 14, 512, 512)
    nc = get_program()

    hx = np.tile(
        (np.arange(FD, dtype=np.float32) - 256.5)[None, :], (P, 1)
    ).astype(np.float32)

    in_maps = []
    mults = []
    for m in range(8):
        b, v = m // 4, m % 4
        cst, mult = _make_consts(
            np.asarray(intrinsics[b], np.float32),
            np.asarray(extrinsics[b, v, :3, :3], np.float32),
            np.asarray(extrinsics[b, v, :3, 3], np.float32),
        )
        mults.append(mult)
        xm = np.ascontiguousarray(
            np.asarray(raw_gaussians[b, v], np.float32).reshape(NCH_IN, NPIX)
        )
        in_maps.append({"x": xm, "hx": hx, "consts": cst})

    global LAST_IN_MAPS
    LAST_IN_MAPS = in_maps
    res = run_bass_kernel_spmd(nc, in_maps, core_ids=list(range(8)))

    means = np.empty((2, 4, 512, 512, 3), np.float32)
    cov = np.empty((2, 4, 512, 512, 3, 3), np.float32)
    opacity = np.empty((2, 4, 512, 512, 1), np.float32)
    rgb = np.empty((2, 4, 512, 512, 3), np.float32)
    big_num = 0.0
    big_cnt = 0.0
    small_num = 0.0
    small_cnt = 0.0
    covmap = {(0, 0): 3, (0, 1): 4, (0, 2): 5, (1, 1): 6, (1, 2): 7, (2, 2): 8}
    for m in range(8):
        b, v = m // 4, m % 4
        o = res.results[m]["out"].reshape(NCH_OUT, 512, 512)
        means[b, v] = np.moveaxis(o[0:3], 0, -1)
        for (i, l), ocn in covmap.items():
            cov[b, v, :, :, i, l] = o[ocn]
            if i != l:
                cov[b, v, :, :, l, i] = o[ocn]
        opacity[b, v] = o[9][..., None]
        rgb[b, v] = np.moveaxis(o[10:13], 0, -1)

        a = res.results[m]["acc"].reshape(P, NCHUNK, 12).sum(axis=(0, 1))
        mult = np.float64(mults[m])
        big_cnt += a[0] + a[1] + a[2]
        big_num += mult * (a[3] + a[4] + a[5]) + 0.05 * (a[0] + a[1] + a[2])
        small_cnt += a[6] + a[7] + a[8]
        small_num += (a[9] + a[10] + a[11]) + G_THR * (a[6] + a[7] + a[8])

    big_loss = np.float32(big_num / big_cnt) if big_cnt > 0 else np.float32(0.0)
    small_loss = (
        np.float32(small_num / small_cnt) if small_cnt > 0 else np.float32(0.0)
    )
    return means, cov, opacity, rgb, big_loss, small_loss


# revision 9
# speedup vs baseline: 1.3898x; 1.1734x over previous
"""Trainium2 Bass kernel for nn_DecoderSplatting.

Contract: kernel(**inputs) takes the FULL unsharded inputs
(raw_gaussians [2,4,14,512,512], extrinsics [2,4,3,4], intrinsics [2,3,3])
and returns the full outputs matching reference():
(means, cov, opacity, rgb, big_loss, small_loss).

Sharding: data-parallel over the 8 (b, v) slices, one per NeuronCore.

All ScalarE ops use the single natural_log_exp_and_others table set
(Exp, Ln, Square, Relu) so no ACT table reloads ever happen:
  sigmoid(x) = 1/(1 + e^-x)      (Exp + DVE fast reciprocal)
  softplus(x) = Ln(e^x + 1)      (Exp + Ln-with-bias)
  1/sqrt(g)   = Exp(-0.5 Ln(g))
"""
import numpy as np

import concourse.bacc as bacc
import concourse.mybir as mybir
from concourse.tile import TileContext
from concourse.bass_utils import run_bass_kernel_spmd

F32 = mybir.dt.float32
ALU = mybir.AluOpType
ACTF = mybir.ActivationFunctionType

P = 128          # partitions
FD = 512         # free-dim chunk size
NCHUNK = 4       # 128*2048 pixels per core, 4 chunks of 512
NPIX = P * 2048  # 262144 pixels per core
NCH_IN = 14
NCH_OUT = 13     # 3 means + 6 cov + 1 opacity + 3 rgb
NACC = 48        # 4 chunks x 12 accumulator columns

G_THR = float(-0.1 * np.log(1e-6))

# const column indices
C_R00, C_R01, C_R10, C_R11, C_R20, C_R21 = 0, 1, 2, 3, 4, 5
C_CM0, C_CM1, C_CM2 = 6, 7, 8          # R[i,2]*f
C_T0, C_T1, C_T2 = 9, 10, 11
C_F2 = 12
C_MULT = 13
C_THRB = 14
C_NTHRB = 15                            # -thrb
C_THRS = 16
C_MC = 17                               # 17..32: Mc row-major
C_HY = 33                               # 33..36: hy per chunk
C_NGTHR = 37
NCOL = 38


class _BaccOneActTable(bacc.Bacc):
    """Restrict ACT table selection to the single set containing every
    activation function this kernel uses (Exp, Ln, Square, Relu), so the
    table-load pass emits exactly one load instead of thrashing."""

    def insert_act_table_loads(self):
        import bass_rust as _br
        from concourse.hw_specs import get_activation_tables
        tables = [
            (n, (f if n == "natural_log_exp_and_others" else set()))
            for n, f in get_activation_tables(self.m.arch).items()
        ]
        _br.insert_act_table_loads(self, tables)


def _build_program(reps: int = 1):
    nc = _BaccOneActTable("TRN2", target_bir_lowering=False, debug=False, num_devices=8)

    x = nc.dram_tensor("x", [NCH_IN, NPIX], F32, kind="ExternalInput")
    hx_d = nc.dram_tensor("hx", [P, FD], F32, kind="ExternalInput")
    cst_d = nc.dram_tensor("consts", [P, NCOL], F32, kind="ExternalInput")
    out = nc.dram_tensor("out", [NCH_OUT, NPIX], F32, kind="ExternalOutput")
    acc_d = nc.dram_tensor("acc", [P, NACC], F32, kind="ExternalOutput")

    # [ch, pix] viewed as [p, ch, f-total]
    x_v = x[:].rearrange("c (p f) -> p c f", p=P)
    out_v = out[:].rearrange("c (p f) -> p c f", p=P)

    with TileContext(nc) as tc:
        with (
            tc.tile_pool(name="cpool", bufs=1) as cp,
            tc.tile_pool(name="iop", bufs=2) as iop,
            tc.tile_pool(name="oop", bufs=2) as oop,
            tc.tile_pool(name="wp", bufs=1) as wp,
            tc.tile_pool(name="scr", bufs=2) as sc,
        ):
            cst = cp.tile([P, NCOL], F32, tag="cst")
            nc.sync.dma_start(out=cst[:], in_=cst_d[:])
            hx = cp.tile([P, FD], F32, tag="hx")
            nc.sync.dma_start(out=hx[:], in_=hx_d[:])
            acc = cp.tile([P, NACC], F32, tag="acc")

            def col(j):
                return cst[:, j:j + 1]

            act = nc.scalar.activation
            tt = nc.vector.tensor_tensor
            ts = nc.vector.tensor_scalar
            stt = nc.vector.scalar_tensor_tensor
            recip = nc.vector.reciprocal_approx_fast

            for c in range(NCHUNK * reps):
                c = c % NCHUNK
                in_t = iop.tile([P, NCH_IN, FD], F32, tag="in_t")
                nc.sync.dma_start(
                    out=in_t[:], in_=x_v[:, :, c * FD:(c + 1) * FD]
                )
                out_t = oop.tile([P, NCH_OUT, FD], F32, tag="out_t")

                def ch(k):
                    return in_t[:, k, :]

                def och(k):
                    return out_t[:, k, :]

                def sigmoid_into(dst, src_ap):
                    den = sc.tile([P, FD], F32, tag="den")
                    act(den[:], src_ap, ACTF.Exp, scale=-1.0)
                    ts(den[:], den[:], 1.0, None, op0=ALU.add)
                    recip(out=dst, in_=den[:])

                def softplus_into(dst, src_ap):
                    e = sc.tile([P, FD], F32, tag="den")
                    act(e[:], src_ap, ACTF.Exp)
                    act(dst, e[:], ACTF.Ln, bias=1.0)

                # ---------------- means ----------------
                s0 = wp.tile([P, FD], F32, tag="s0")
                sigmoid_into(s0[:], ch(12))
                s1 = wp.tile([P, FD], F32, tag="s1")
                sigmoid_into(s1[:], ch(13))
                tt(out=s0[:], in0=s0[:], in1=hx[:], op=ALU.add)          # w0
                ts(s1[:], s1[:], col(C_HY + c), None, op0=ALU.add)       # w1
                sq0 = wp.tile([P, FD], F32, tag="sq0")
                act(sq0[:], s0[:], ACTF.Square)
                sq1 = wp.tile([P, FD], F32, tag="sq1")
                act(sq1[:], s1[:], ACTF.Square)
                # nnf = (w0^2 + f^2) + w1^2
                stt(sq0[:], sq0[:], col(C_F2), sq1[:], op0=ALU.add, op1=ALU.add)
                tex = wp.tile([P, FD], F32, tag="tex")
                act(tex[:], ch(3), ACTF.Exp, scale=-1.0)                 # e^-disp
                gg = wp.tile([P, FD], F32, tag="gg")
                act(gg[:], tex[:], ACTF.Square, bias=1.0, scale=0.01)    # ((t+100)/100)^2
                tt(out=gg[:], in0=gg[:], in1=sq0[:], op=ALU.mult)        # g
                act(sq1[:], gg[:], ACTF.Ln)                              # ln g
                act(gg[:], sq1[:], ACTF.Exp, scale=-0.5)                 # g^-1/2
                # sfac = (t+1) * g^-1/2
                stt(sq1[:], tex[:], 1.0, gg[:], op0=ALU.add, op1=ALU.mult)
                tt(out=s0[:], in0=s0[:], in1=sq1[:], op=ALU.mult)        # W0
                tt(out=s1[:], in0=s1[:], in1=sq1[:], op=ALU.mult)        # W1
                for i in range(3):
                    xi = sc.tile([P, FD], F32, tag="xi")
                    ts(xi[:], sq1[:], col(C_CM0 + i), col(C_T0 + i),
                       op0=ALU.mult, op1=ALU.add)
                    stt(xi[:], s0[:], col(C_R00 + 2 * i), xi[:],
                        op0=ALU.mult, op1=ALU.add)
                    stt(och(i), s1[:], col(C_R01 + 2 * i), xi[:],
                        op0=ALU.mult, op1=ALU.add)

                # ---------------- opacity / rgb ----------------
                sigmoid_into(och(9), ch(4))
                for i in range(3):
                    softplus_into(och(10 + i), ch(i))

                # ---------------- scales + losses ----------------
                sp = []
                for k in range(3):
                    spk = wp.tile([P, FD], F32, tag=f"sp{k}")
                    softplus_into(spk[:], ch(5 + k))
                    sp.append(spk)
                a0 = c * 12
                for k in range(3):
                    junk = sc.tile([P, FD], F32, tag="junk")
                    ts(junk[:], sp[k][:], col(C_THRB), 0.0, op0=ALU.is_gt,
                       op1=ALU.add, accum_out=acc[:, a0 + k:a0 + k + 1])
                    junk = sc.tile([P, FD], F32, tag="junk")
                    act(junk[:], sp[k][:], ACTF.Relu, bias=col(C_NTHRB),
                        accum_out=acc[:, a0 + 3 + k:a0 + 4 + k])
                    junk = sc.tile([P, FD], F32, tag="junk")
                    ts(junk[:], sp[k][:], col(C_THRS), 0.0, op0=ALU.is_lt,
                       op1=ALU.add, accum_out=acc[:, a0 + 6 + k:a0 + 7 + k])
                    lnt = sc.tile([P, FD], F32, tag="lnt")
                    act(lnt[:], sp[k][:], ACTF.Ln, scale=col(C_MULT))
                    junk = sc.tile([P, FD], F32, tag="junk")
                    act(junk[:], lnt[:], ACTF.Relu, bias=col(C_NGTHR), scale=-0.1,
                        accum_out=acc[:, a0 + 9 + k:a0 + 10 + k])

                # ---------------- quaternion -> q' = Mc q ----------------
                qp = []
                for a in range(4):
                    qpa = wp.tile([P, FD], F32, tag=f"qp{a}")
                    nc.gpsimd.tensor_scalar(
                        qpa[:], ch(8), col(C_MC + 4 * a), None, op0=ALU.mult)
                    for bq in range(1, 4):
                        nc.gpsimd.scalar_tensor_tensor(
                            qpa[:], ch(8 + bq), col(C_MC + 4 * a + bq), qpa[:],
                            op0=ALU.mult, op1=ALU.add)
                    qp.append(qpa)

                # squares
                sqq = []
                for a in range(4):
                    t = wp.tile([P, FD], F32, tag=f"qq{a}")
                    act(t[:], qp[a][:], ACTF.Square)
                    sqq.append(t)
                A, B, C_, D = sqq

                # doubled cross products
                d2 = sc.tile([P, FD], F32, tag="d2")
                nc.gpsimd.tensor_scalar(d2[:], qp[0][:], 2.0, None, op0=ALU.mult)
                ab = wp.tile([P, FD], F32, tag="ab")
                nc.gpsimd.tensor_tensor(out=ab[:], in0=d2[:], in1=qp[1][:], op=ALU.mult)
                acx = wp.tile([P, FD], F32, tag="acx")
                nc.gpsimd.tensor_tensor(out=acx[:], in0=d2[:], in1=qp[2][:], op=ALU.mult)
                ad = wp.tile([P, FD], F32, tag="ad")
                nc.gpsimd.tensor_tensor(out=ad[:], in0=d2[:], in1=qp[3][:], op=ALU.mult)
                d2 = sc.tile([P, FD], F32, tag="d2")
                nc.gpsimd.tensor_scalar(d2[:], qp[1][:], 2.0, None, op0=ALU.mult)
                bc = wp.tile([P, FD], F32, tag="bc")
                nc.gpsimd.tensor_tensor(out=bc[:], in0=d2[:], in1=qp[2][:], op=ALU.mult)
                bd = wp.tile([P, FD], F32, tag="bd")
                nc.gpsimd.tensor_tensor(out=bd[:], in0=d2[:], in1=qp[3][:], op=ALU.mult)
                d2 = sc.tile([P, FD], F32, tag="d2")
                nc.gpsimd.tensor_scalar(d2[:], qp[2][:], 2.0, None, op0=ALU.mult)
                cd = wp.tile([P, FD], F32, tag="cd")
                nc.gpsimd.tensor_tensor(out=cd[:], in0=d2[:], in1=qp[3][:], op=ALU.mult)

                # off-diagonal P entries.  P10 overwrites ab, P20 overwrites
                # acx, P21 overwrites bc (their last reads happen first).
                P01 = wp.tile([P, FD], F32, tag="P01")
                tt(out=P01[:], in0=ab[:], in1=cd[:], op=ALU.subtract)
                tt(out=ab[:], in0=ab[:], in1=cd[:], op=ALU.add)            # P10
                P02 = wp.tile([P, FD], F32, tag="P02")
                tt(out=P02[:], in0=acx[:], in1=bd[:], op=ALU.add)
                tt(out=acx[:], in0=acx[:], in1=bd[:], op=ALU.subtract)     # P20
                P12 = wp.tile([P, FD], F32, tag="P12")
                tt(out=P12[:], in0=bc[:], in1=ad[:], op=ALU.subtract)
                tt(out=bc[:], in0=bc[:], in1=ad[:], op=ALU.add)            # P21
                P10, P20, P21 = ab, acx, bc

                # diagonal: u=A+D (into D), vv=B+C (into cd), n2e (into ad),
                # P00 (into bd), P11 (into C_), P22 (into B)
                tt(out=D[:], in0=A[:], in1=D[:], op=ALU.add)               # u
                tt(out=cd[:], in0=B[:], in1=C_[:], op=ALU.add)             # vv
                stt(ad[:], D[:], 1e-8, cd[:], op0=ALU.add, op1=ALU.add)    # n2e
                stt(bd[:], cd[:], -2.0, ad[:], op0=ALU.mult, op1=ALU.add)  # P00
                tt(out=D[:], in0=A[:], in1=C_[:], op=ALU.add)              # v2
                stt(C_[:], D[:], -2.0, ad[:], op0=ALU.mult, op1=ALU.add)   # P11->C_
                tt(out=D[:], in0=A[:], in1=B[:], op=ALU.add)               # v3
                stt(B[:], D[:], -2.0, ad[:], op0=ALU.mult, op1=ALU.add)    # P22->B
                P00, P11, P22 = bd, C_, B

                # h = 1/n2e (into A), mh = h*mult (in place)
                recip(out=A[:], in_=ad[:])
                ts(A[:], A[:], col(C_MULT), None, op0=ALU.mult)            # mh

                # sz_k = sp_k * mh  (in place into sp_k)
                for k in range(3):
                    tt(out=sp[k][:], in0=sp[k][:], in1=A[:], op=ALU.mult)

                # Y[i][k] = P[i][k] * sz_k  (in place into P tiles)
                Pm = [[P00, P01, P02], [P10, P11, P12], [P20, P21, P22]]
                for i in range(3):
                    for k in range(3):
                        tt(out=Pm[i][k][:], in0=Pm[i][k][:], in1=sp[k][:],
                           op=ALU.mult)
                Y = Pm

                # cov output channel map: 3:c00 4:c01 5:c02 6:c11 7:c12 8:c22
                # diagonals via ACT squares
                for i, oc in ((0, 3), (1, 6), (2, 8)):
                    ya = sc.tile([P, FD], F32, tag="ya")
                    act(ya[:], Y[i][0][:], ACTF.Square)
                    yb = sc.tile([P, FD], F32, tag="yb")
                    act(yb[:], Y[i][1][:], ACTF.Square)
                    yc = sc.tile([P, FD], F32, tag="yc")
                    act(yc[:], Y[i][2][:], ACTF.Square)
                    tt(out=ya[:], in0=ya[:], in1=yb[:], op=ALU.add)
                    tt(out=och(oc), in0=ya[:], in1=yc[:], op=ALU.add)
                # off-diagonals
                for i, l, oc in ((0, 1, 4), (0, 2, 5), (1, 2, 7)):
                    e1 = sc.tile([P, FD], F32, tag="e1")
                    tt(out=e1[:], in0=Y[i][0][:], in1=Y[l][0][:], op=ALU.mult)
                    e2 = sc.tile([P, FD], F32, tag="e2")
                    tt(out=e2[:], in0=Y[i][1][:], in1=Y[l][1][:], op=ALU.mult)
                    tt(out=e1[:], in0=e1[:], in1=e2[:], op=ALU.add)
                    tt(out=e2[:], in0=Y[i][2][:], in1=Y[l][2][:], op=ALU.mult)
                    tt(out=och(oc), in0=e1[:], in1=e2[:], op=ALU.add)

                nc.sync.dma_start(
                    out=out_v[:, :, c * FD:(c + 1) * FD], in_=out_t[:]
                )

            nc.sync.dma_start(out=acc_d[:], in_=acc[:])

    nc.compile()
    return nc


_CACHED_NC = {}
LAST_IN_MAPS = None


def get_program(reps: int = 1):
    if reps not in _CACHED_NC:
        _CACHED_NC[reps] = _build_program(reps)
    return _CACHED_NC[reps]


def _rot_to_quat(R):
    R = R.astype(np.float64)
    t = np.trace(R)
    if t > 0:
        s = np.sqrt(t + 1.0) * 2
        w = 0.25 * s
        xq = (R[2, 1] - R[1, 2]) / s
        y = (R[0, 2] - R[2, 0]) / s
        z = (R[1, 0] - R[0, 1]) / s
    else:
        i = int(np.argmax(np.diag(R)))
        if i == 0:
            s = np.sqrt(1.0 + R[0, 0] - R[1, 1] - R[2, 2]) * 2
            xq = 0.25 * s
            w = (R[2, 1] - R[1, 2]) / s
            y = (R[0, 1] + R[1, 0]) / s
            z = (R[0, 2] + R[2, 0]) / s
        elif i == 1:
            s = np.sqrt(1.0 - R[0, 0] + R[1, 1] - R[2, 2]) * 2
            y = 0.25 * s
            w = (R[0, 2] - R[2, 0]) / s
            xq = (R[0, 1] + R[1, 0]) / s
            z = (R[1, 2] + R[2, 1]) / s
        else:
            s = np.sqrt(1.0 - R[0, 0] - R[1, 1] + R[2, 2]) * 2
            z = 0.25 * s
            w = (R[1, 0] - R[0, 1]) / s
            xq = (R[0, 2] + R[2, 0]) / s
            y = (R[1, 2] + R[2, 1]) / s
    q = np.array([xq, y, z, w])
    return q / np.linalg.norm(q)


def _quat_lmul(qc):
    xq, y, z, w = qc
    return np.array([
        [w, -z, y, xq],
        [z, w, -xq, y],
        [-y, xq, w, z],
        [-xq, -y, -z, w],
    ])


def _make_consts(K, R, t):
    f = np.float64(K[0, 0])
    mult = np.float64(np.linalg.inv(K[:2, :2].astype(np.float64)).sum())
    qc = _rot_to_quat(R)
    Mc = _quat_lmul(qc)
    cols = np.zeros(NCOL, np.float64)
    cols[C_R00], cols[C_R01] = R[0, 0], R[0, 1]
    cols[C_R10], cols[C_R11] = R[1, 0], R[1, 1]
    cols[C_R20], cols[C_R21] = R[2, 0], R[2, 1]
    for i in range(3):
        cols[C_CM0 + i] = np.float64(R[i, 2]) * f
        cols[C_T0 + i] = t[i]
    cols[C_F2] = f * f
    cols[C_MULT] = mult
    thrb = np.float32(0.05 / np.float32(mult))
    thrs = np.float32(1e-6 / np.float32(mult))
    cols[C_THRB] = thrb
    cols[C_NTHRB] = -thrb
    cols[C_THRS] = thrs
    cols[C_NGTHR] = -G_THR
    for a in range(4):
        for bq in range(4):
            cols[C_MC + 4 * a + bq] = Mc[a, bq]
    cst = np.tile(cols.astype(np.float32)[None, :], (P, 1))
    # hy columns vary per partition: hy[p, c] = 4p + c - 256.5
    pp = np.arange(P, dtype=np.float32)
    for c in range(NCHUNK):
        cst[:, C_HY + c] = 4.0 * pp + c - 256.5
    return cst, np.float32(mult)


def kernel(raw_gaussians, extrinsics, intrinsics):
    b_, v_, c_, h_, w_ = raw_gaussians.shape
    assert (b_, v_, c_, h_, w_) == (2, 4, 14, 512, 512)
    nc = get_program()

    hx = np.tile(
        (np.arange(FD, dtype=np.float32) - 256.5)[None, :], (P, 1)
    ).astype(np.float32)

    in_maps = []
    mults = []
    for m in range(8):
        b, v = m // 4, m % 4
        cst, mult = _make_consts(
            np.asarray(intrinsics[b], np.float32),
            np.asarray(extrinsics[b, v, :3, :3], np.float32),
            np.asarray(extrinsics[b, v, :3, 3], np.float32),
        )
        mults.append(mult)
        xm = np.ascontiguousarray(
            np.asarray(raw_gaussians[b, v], np.float32).reshape(NCH_IN, NPIX)
        )
        in_maps.append({"x": xm, "hx": hx, "consts": cst})

    global LAST_IN_MAPS
    LAST_IN_MAPS = in_maps
    res = run_bass_kernel_spmd(nc, in_maps, core_ids=list(range(8)))

    means = np.empty((2, 4, 512, 512, 3), np.float32)
    cov = np.empty((2, 4, 512, 512, 3, 3), np.float32)
    opacity = np.empty((2, 4, 512, 512, 1), np.float32)
    rgb = np.empty((2, 4, 512, 512, 3), np.float32)
    big_num = 0.0
    big_cnt = 0.0
    small_num = 0.0
    small_cnt = 0.0
    covmap = {(0, 0): 3, (0, 1): 4, (0, 2): 5, (1, 1): 6, (1, 2): 7, (2, 2): 8}
    for m in range(8):
        b, v = m // 4, m % 4
        o = res.results[m]["out"].reshape(NCH_OUT, 512, 512)
        means[b, v] = np.moveaxis(o[0:3], 0, -1)
        for (i, l), ocn in covmap.items():
            cov[b, v, :, :, i, l] = o[ocn]
            if i != l:
                cov[b, v, :, :, l, i] = o[ocn]
        opacity[b, v] = o[9][..., None]
        rgb[b, v] = np.moveaxis(o[10:13], 0, -1)

        a = res.results[m]["acc"].reshape(P, NCHUNK, 12).sum(axis=(0, 1))
        mult = np.float64(mults[m])
        big_cnt += a[0] + a[1] + a[2]
        big_num += mult * (a[3] + a[4] + a[5]) + 0.05 * (a[0] + a[1] + a[2])
        small_cnt += a[6] + a[7] + a[8]
        small_num += (a[9] + a[10] + a[11]) + G_THR * (a[6] + a[7] + a[8])

    big_loss = np.float32(big_num / big_cnt) if big_cnt > 0 else np.float32(0.0)
    small_loss = (
        np.float32(small_num / small_cnt) if small_cnt > 0 else np.float32(0.0)
    )
    return means, cov, opacity, rgb, big_loss, small_loss
